# revision 5
# baseline (speedup 1.0000x reference)
"""Trainium2 Bass kernel for nn_ChemResBlock (gnn_message_passing).

Reference computation (A=2048 atoms, D=64 depth, F=12 filter slots):
    chemconv(x)[a,o] = sum_{n,f,d} conn[a,n,f] * x[n,d] * filters[o,f,d]
                       + sum_{f,c} bond[a,f,c] * filters[o,f,D+c]
    for filt in (f0, f1):
        out = relu(chemconv(out)); out = chemconv(out); out = relu(out + x)

Kernel strategy (8 NeuronCores):
  * Contract-reorder: out[a,o] = sum_{n,f} conn[a,n,f] * y[n,f,o] with
    y[n,f,o] = sum_d x[n,d]*filters[o,f,d]  (tiny per-shard precompute), so
    the big conn tensor is consumed by plain [128,64]x[128,512] matmuls.
  * Shard the contraction (neighbor) dim n across 8 cores.  Core c owns the
    non-contiguous atom set {c*128..(c+1)*128} u {1024+c*128..1024+(c+1)*128}
    so each half of the output columns can be Reduce-Scattered separately.
  * fp16 everywhere on the big path: conn is cast to fp16 on the host
    (12 MiB/core, ALL 24 k-chunks SBUF-resident, read from HBM once, split
    across two DMA rings), y is fp16, the ReduceScatter payload is fp16.
    Activations grow ~200x per conv (absmax 5.7e8 by conv3), so each conv's
    y is pre-scaled by a power-of-2 (1, 1, 2^-6, 2^-14); the fp32
    elementwise stage unscales before bias/residual/relu.  Measured
    absmax/scale error vs the fp32 reference: ~1.9e-3 (gate 2e-2).
  * Per conn chunk the y-slice weights load once and 2 (or 4) matmuls
    accumulate into separate psum banks.  tile_legalize splits each fp16
    matmul into LDWEIGHTS+MATMUL; a post-schedule strip pass removes the
    redundant (same-AP, syncless) LDWEIGHTS.
  * Split-RS pipeline: convs 1-3 run the big matmul in two column-half
    passes (A = cols 0..1023, B = 1024..2047), each followed by its own
    half-size fp16 ReduceScatter.  RS_A overlaps pass B; RS_B overlaps the
    next conv's first 12 chunks (which only need y rows from the ew-A
    half).  conv0 keeps a single 4-bank pass (it is paced by the conn DMA
    stream anyway) followed by both RS.
"""

import os

import numpy as np

import concourse.bacc as bacc
import concourse.bass as bass
import concourse.mybir as mybir
import concourse.tile as tile
from concourse.bass_utils import run_bass_kernel_spmd

A, D, F, NCORES = 2048, 64, 12, 8
NS = A // NCORES          # neighbors per core = 256
HB = NS // 2              # half-block = 128 columns owned per RS half
KL = NS * F               # local contraction size = 3072
NCH = KL // 128           # k-chunks of 128 = 24
ABLK = 512                # output free-dim block (psum bank)
NAB = A // ABLK           # 4
FO = F * D                # 768 = y columns per layer

FP = mybir.dt.float32
F16 = mybir.dt.float16

# per-conv y scales (power of 2): keep fp16-cast activations in range
SCALES = [1.0, 1.0, 2.0 ** -6, 2.0 ** -14]

STRIP = os.environ.get("CHEM_STRIP", "1") == "1"

_CACHE = {}

_PE = mybir.EngineType.PE
ACT_COPY = mybir.ActivationFunctionType.Copy


def _strip_redundant_ldweights(nc):
    """Remove LDWEIGHTS that reload the already-loaded stationary AP.

    tile_legalize splits every non-f32 InstMatmult into InstLdweights +
    non-self-loading InstMatmult.  Consecutive matmuls that share weights
    get one redundant load per matmul; those extra loads carry no sync
    info and can be dropped before nc.compile() (whose
    move_matmul_waits_to_ldweights pass then attaches matmul waits to the
    surviving loads)."""
    removed = 0
    for f in nc.m.functions:
        for blk in f.blocks:
            cur_ap = None
            kept = []
            for inst in blk.instructions:
                tn = type(inst).__name__
                if getattr(inst, "engine", None) == _PE:
                    if tn == "InstLdweights":
                        ap = str(inst.ins[0])
                        si = inst.sync_info
                        clean = si is None or (not si.on_wait and not si.on_update)
                        if clean and ap == cur_ap:
                            removed += 1
                            continue
                        cur_ap = ap
                    elif tn == "InstMatmult":
                        if inst.ldweights is not False:
                            cur_ap = None  # self-loading matmul clobbers PE
                    elif tn in ("InstEventSemaphore", "InstDrain", "InstISA",
                                "InstTensorLoad", "InstTensorSave"):
                        pass
                    else:
                        cur_ap = None
                kept.append(inst)
            if removed:
                blk.instructions = kept
    return removed


def _build():
    nc = bacc.Bacc("TRN2", target_bir_lowering=False, debug=False, num_devices=NCORES)

    conn_t_d = nc.dram_tensor("conn_t", [KL, A], F16, kind="ExternalInput").ap()
    xoT_d = nc.dram_tensor("xoT_sh", [D, NS], FP, kind="ExternalInput").ap()
    xoT16_d = nc.dram_tensor("xoT16_sh", [D, NS], F16, kind="ExternalInput").ap()
    fw_d = nc.dram_tensor("fw16", [D, 2 * FO], F16, kind="ExternalInput").ap()
    fb_d = nc.dram_tensor("fb", [2 * F, 2 * D], FP, kind="ExternalInput").ap()
    bondT_d = nc.dram_tensor("bondT_sh", [2 * F, NS], FP, kind="ExternalInput").ap()
    out_d = nc.dram_tensor("out_sh", [D, NS], FP, kind="ExternalOutput").ap()

    with tile.TileContext(nc) as tc:
        with (
            tc.tile_pool(name="res", bufs=1) as res_pool,
            tc.tile_pool(name="sb", bufs=1) as sb,
            tc.tile_pool(name="ypool", bufs=2) as ypool,
            tc.tile_pool(name="ztpool", bufs=4) as ztpool,
            tc.tile_pool(name="work", bufs=2) as work,
            tc.tile_pool(name="psy", bufs=2, space="PSUM") as psy,
            tc.tile_pool(name="psz", bufs=1, space="PSUM") as psz,
            tc.tile_pool(name="dram", bufs=1, space="DRAM") as dram,
        ):
            # ---- setup: small tensors first (ACT HWDGE ring), then conn
            # chunks split across the SP + POOL rings so conv0 can start
            # while conn streams in ----
            xoT16_sb = sb.tile([D, NS], F16, name="xoT16_sb", tag="xoT16_sb")
            nc.scalar.dma_start(xoT16_sb[:], xoT16_d)
            fw_sb = sb.tile([D, 2 * FO], F16, name="fw_sb", tag="fw_sb")
            nc.scalar.dma_start(fw_sb[:], fw_d)
            xoT_sb = sb.tile([D, NS], FP, name="xoT_sb", tag="xoT_sb")
            nc.scalar.dma_start(xoT_sb[:], xoT_d)
            fb_sb = sb.tile([2 * F, 2 * D], FP, name="fb_sb", tag="fb_sb")
            nc.scalar.dma_start(fb_sb[:], fb_d)
            bondT_sb = sb.tile([2 * F, NS], FP, name="bondT_sb", tag="bondT_sb")
            nc.scalar.dma_start(bondT_sb[:], bondT_d)

            conn_res = []
            for r in range(NCH):
                t = res_pool.tile([128, A], F16, name=f"connsb{r}", tag=f"connsb{r}")
                eng = nc.gpsimd if r % 2 == 1 else nc.sync
                eng.dma_start(t[:], conn_t_d[r * 128:(r + 1) * 128, :])
                conn_res.append(t)

            # per-layer bias shard: bias[l][o, a_local] (fp32, true scale)
            bias_sb = sb.tile([D, 2, NS], FP, name="bias_sb", tag="bias_sb")
            for layer in range(2):
                pb = psy.tile([D, NS], FP, name="pb", tag="py")
                nc.tensor.matmul(
                    pb[:], fb_sb[:, layer * D:(layer + 1) * D], bondT_sb[:],
                    start=True, stop=True,
                )
                nc.vector.tensor_copy(bias_sb[:, layer, :], pb[:])

            cc_in = {}
            cc_out = {}
            for i in range(4):
                for hf in "AB":
                    cc_in[i, hf] = dram.tile(
                        [NCORES, D, HB], F16, name=f"cc_in{hf}{i}", tag=f"cc_in{hf}{i}")
                    cc_out[i, hf] = dram.tile(
                        [D, HB], F16, name=f"cc_out{hf}{i}", tag=f"cc_out{hf}{i}")

            scope = nc.named_scope

            def y_mm(y_sb, cur16, layer, ns_):
                """y rows for n-block ns_ (both fo-halves)."""
                for h in range(2):
                    py = psy.tile([128, FO // 2], FP, name="py", tag="py")
                    nc.tensor.matmul(
                        py[:],
                        cur16[:, ns_ * 128:(ns_ + 1) * 128],
                        fw_sb[:, layer * FO + h * (FO // 2):
                              layer * FO + (h + 1) * (FO // 2)],
                        start=True, stop=True,
                    )
                    nc.vector.tensor_copy(
                        y_sb[:, ns_, h * (FO // 2):(h + 1) * (FO // 2)], py[:]
                    )

            def drain(conv, hf, pz0, pz1):
                """Cast two psum banks (one column half) and DMA to cc_in."""
                zts = []
                for bi, pzb in enumerate((pz0, pz1)):
                    zt = ztpool.tile([D, ABLK], F16, name="zt", tag="zt")
                    if bi == 0:
                        nc.vector.tensor_copy(zt[:], pzb[:])
                    else:
                        nc.scalar.activation(zt[:], pzb[:], ACT_COPY)
                    zts.append(zt)
                for bi, zt in enumerate(zts):
                    for j in range(4):
                        eng = nc.scalar if j % 2 == 0 else nc.sync
                        eng.dma_start(
                            cc_in[conv, hf][4 * bi + j, :, :],
                            zt[:, j * HB:(j + 1) * HB],
                        )

            def rs(conv, hf):
                scc = scope(f"cc{conv}{hf}"); scc.__enter__()
                nc.gpsimd.collective_compute(
                    "ReduceScatter",
                    mybir.AluOpType.add,
                    replica_groups=[list(range(NCORES))],
                    ins=[cc_in[conv, hf].opt()],
                    outs=[cc_out[conv, hf].opt()],
                )
                scc.__exit__(None, None, None)

            def elementwise(conv, hf, layer, nxt, c16):
                """unscale + bias + residual + relu for one 128-col half."""
                hh = 0 if hf == "A" else 1
                hs = slice(hh * HB, (hh + 1) * HB)
                inv_s = 1.0 / SCALES[conv]
                sl = work.tile([D, HB], F16, name=f"sl{hf}", tag=f"sl{hf}")
                nc.scalar.dma_start(sl[:], cc_out[conv, hf])
                t1 = work.tile([D, HB], FP, name=f"t1{hf}", tag=f"t1{hf}")
                nc.vector.tensor_scalar_mul(t1[:], sl[:], inv_s)
                t2 = work.tile([D, HB], FP, name=f"t2{hf}", tag=f"t2{hf}")
                nc.vector.tensor_add(t2[:], t1[:], bias_sb[:, layer, hs])
                if conv % 2 == 1:
                    t3 = work.tile([D, HB], FP, name=f"t3{hf}", tag=f"t3{hf}")
                    nc.vector.tensor_add(t3[:], t2[:], xoT_sb[:, hs])
                    t2 = t3
                nc.vector.tensor_scalar_max(nxt[:, hs], t2[:], 0.0)
                if conv < 3:
                    nc.vector.tensor_scalar_mul(
                        c16[:, hs], nxt[:, hs], SCALES[conv + 1])
                else:
                    nc.scalar.dma_start(out_d[:, hs], nxt[:, hs])

            # ---------------- conv 0: single pass, paced by conn DMA -------
            sc = scope("conv0"); sc.__enter__()
            y_sb = ypool.tile([128, 2, FO], F16, name="y_sb", tag="y_sb")
            y_mm(y_sb, xoT16_sb, 0, 0)
            y_mm(y_sb, xoT16_sb, 0, 1)
            pz = [psz.tile([D, ABLK], FP, name="pz", tag=f"pz{t_}")
                  for t_ in ("A0", "A1", "B0", "B1")]
            for r in range(NCH):
                f_, ns_ = r // 2, r % 2
                lhsT = y_sb[:, ns_, f_ * D:(f_ + 1) * D]
                for ab in range(NAB):
                    nc.tensor.matmul(
                        pz[ab][:], lhsT,
                        conn_res[r][:, ab * ABLK:(ab + 1) * ABLK],
                        start=(r == 0), stop=(r == NCH - 1),
                    )
            drain(0, "A", pz[0], pz[1])
            sc.__exit__(None, None, None)
            rs(0, "A")
            sc = scope("conv0d"); sc.__enter__()
            drain(0, "B", pz[2], pz[3])
            sc.__exit__(None, None, None)
            rs(0, "B")

            # ---------------- convs 1-3: two passes + split RS -------------
            prev_nxt = work.tile([D, NS], FP, name="nxt", tag="nxt")
            prev_c16 = work.tile([D, NS], F16, name="c16", tag="c16")
            elementwise(0, "A", 0, prev_nxt, prev_c16)

            for conv in range(1, 4):
                layer = conv // 2
                sc = scope(f"conv{conv}"); sc.__enter__()
                y_sb = ypool.tile([128, 2, FO], F16, name="y_sb", tag="y_sb")
                y_mm(y_sb, prev_c16, layer, 0)

                pzA = [psz.tile([D, ABLK], FP, name="pz", tag=f"pzA{b_}")
                       for b_ in range(2)]
                pzB = [psz.tile([D, ABLK], FP, name="pz", tag=f"pzB{b_}")
                       for b_ in range(2)]

                # pass A, chunks with ns=0 first (only need y half 0)
                for ci, (ns_, f_) in enumerate(
                        [(n, f) for n in range(2) for f in range(F)]):
                    if ci == F:
                        # before the ns=1 chunks: finish the previous conv's
                        # second half + this conv's y rows for n-block 1
                        elementwise(conv - 1, "B", (conv - 1) // 2,
                                    prev_nxt, prev_c16)
                        y_mm(y_sb, prev_c16, layer, 1)
                    r = 2 * f_ + ns_
                    lhsT = y_sb[:, ns_, f_ * D:(f_ + 1) * D]
                    for ab in range(2):
                        nc.tensor.matmul(
                            pzA[ab][:], lhsT,
                            conn_res[r][:, ab * ABLK:(ab + 1) * ABLK],
                            start=(ci == 0), stop=(ci == NCH - 1),
                        )
                drain(conv, "A", pzA[0], pzA[1])
                sc.__exit__(None, None, None)
                rs(conv, "A")

                sc = scope(f"conv{conv}b"); sc.__enter__()
                # pass B
                for ci, (ns_, f_) in enumerate(
                        [(n, f) for n in range(2) for f in range(F)]):
                    r = 2 * f_ + ns_
                    lhsT = y_sb[:, ns_, f_ * D:(f_ + 1) * D]
                    for ab in range(2):
                        nc.tensor.matmul(
                            pzB[ab][:], lhsT,
                            conn_res[r][:, (2 + ab) * ABLK:(3 + ab) * ABLK],
                            start=(ci == 0), stop=(ci == NCH - 1),
                        )
                drain(conv, "B", pzB[0], pzB[1])
                sc.__exit__(None, None, None)
                rs(conv, "B")

                nxt = work.tile([D, NS], FP, name="nxt", tag="nxt")
                c16 = (work.tile([D, NS], F16, name="c16", tag="c16")
                       if conv < 3 else None)
                elementwise(conv, "A", layer, nxt, c16)
                prev_nxt, prev_c16 = nxt, c16

            # tail: second half of conv3
            elementwise(3, "B", 1, prev_nxt, None)

    if STRIP:
        n = _strip_redundant_ldweights(nc)
        # conv0: 24 chunks x 3; convs 1-3: 48 chunks x 1 each = 216 total,
        # minus the few that carry sync waits and must stay
        # (+8: the two fo-halves of each y_mm call share lhsT weights)
        expect = NCH * 3 + 3 * 2 * NCH + 8
        assert expect - 24 <= n <= expect, f"stripped {n} ldweights"
    nc.compile()
    return nc


def _get_nc():
    if "nc" not in _CACHE:
        _CACHE["nc"] = _build()
    return _CACHE["nc"]


def _own_idx(c):
    return np.r_[c * HB:(c + 1) * HB, A // 2 + c * HB:A // 2 + (c + 1) * HB]


def _prep_in_maps(node_property_tensor, connectivity_tensor, bond_property_tensor,
                  filters0, filters1):
    x = np.ascontiguousarray(node_property_tensor, dtype=np.float32)
    conn = np.ascontiguousarray(connectivity_tensor, dtype=np.float32)
    bond = np.ascontiguousarray(bond_property_tensor, dtype=np.float32)
    f0 = np.ascontiguousarray(filters0, dtype=np.float32)
    f1 = np.ascontiguousarray(filters1, dtype=np.float32)

    # host-side layout transforms (pure transpose/reshape/slice/cast)
    xT = np.ascontiguousarray(x.T)                                   # [D, A]
    xT16 = (xT * SCALES[0]).astype(np.float16)
    fw = np.concatenate(
        [f[:, :, :D].transpose(2, 1, 0).reshape(D, FO) for f in (f0, f1)], axis=1
    ).astype(np.float16)                                             # [D, 2*FO]
    fw = np.ascontiguousarray(fw)
    fb = np.concatenate(
        [f[:, :, D:].reshape(D, 2 * F).T for f in (f0, f1)], axis=1
    )                                                                # [2F, 2D]
    fb = np.ascontiguousarray(fb)
    bondT = np.ascontiguousarray(bond.transpose(1, 2, 0).reshape(2 * F, A))
    conn16 = conn.astype(np.float16)

    in_maps = []
    for c in range(NCORES):
        idx = _own_idx(c)
        conn_t = np.ascontiguousarray(
            conn16[:, idx, :].transpose(2, 1, 0).reshape(KL, A)
        )
        in_maps.append({
            "conn_t": conn_t,
            "xoT_sh": np.ascontiguousarray(xT[:, idx]),
            "xoT16_sh": np.ascontiguousarray(xT16[:, idx]),
            "fw16": fw,
            "fb": fb,
            "bondT_sh": np.ascontiguousarray(bondT[:, idx]),
        })
    return in_maps


def kernel(node_property_tensor, connectivity_tensor, bond_property_tensor,
           filters0, filters1):
    in_maps = _prep_in_maps(node_property_tensor, connectivity_tensor,
                            bond_property_tensor, filters0, filters1)
    nc = _get_nc()
    res = run_bass_kernel_spmd(nc, in_maps, core_ids=list(range(NCORES)))
    outT = np.empty((D, A), dtype=np.float32)
    for c in range(NCORES):
        outT[:, _own_idx(c)] = res.results[c]["out_sh"]
    return np.ascontiguousarray(outT.T)


def run_traced(in_maps, stitch=False):
    """For test.py: run with NTFF tracing, return BassKernelResults."""
    kw = {}
    if stitch:
        kw = dict(trace_cores=list(range(NCORES)), stitch_traces=True)
    return run_bass_kernel_spmd(
        _get_nc(), in_maps, core_ids=list(range(NCORES)), trace=True, **kw
    )


def make_in_maps(**inputs):
    """Expose the host-side prep for test.py tracing path."""
    return _prep_in_maps(
        inputs["node_property_tensor"], inputs["connectivity_tensor"],
        inputs["bond_property_tensor"], inputs["filters0"], inputs["filters1"])


# revision 9
# speedup vs baseline: 1.0190x; 1.0190x over previous
"""Trainium2 Bass kernel for nn_ChemResBlock (gnn_message_passing).

Reference computation (A=2048 atoms, D=64 depth, F=12 filter slots):
    chemconv(x)[a,o] = sum_{n,f,d} conn[a,n,f] * x[n,d] * filters[o,f,d]
                       + sum_{f,c} bond[a,f,c] * filters[o,f,D+c]
    for filt in (f0, f1):
        out = relu(chemconv(out)); out = chemconv(out); out = relu(out + x)

Kernel strategy (8 NeuronCores):
  * Contract-reorder: out[a,o] = sum_{n,f} conn[a,n,f] * y[n,f,o] with
    y[n,f,o] = sum_d x[n,d]*filters[o,f,d]  (tiny per-shard precompute), so
    the big conn tensor is consumed by plain [128,64]x[128,512] matmuls.
  * Shard the contraction (neighbor) dim n across 8 cores.  Core c owns the
    non-contiguous atom set {c*128..(c+1)*128} u {1024+c*128..1024+(c+1)*128}
    so each half of the output columns can be Reduce-Scattered separately.
  * fp16 everywhere on the big path: conn is cast to fp16 on the host
    (12 MiB/core, ALL 24 k-chunks SBUF-resident, read from HBM once, split
    across two DMA rings), y is fp16, the ReduceScatter payload is fp16.
    Activations grow ~200x per conv (absmax 5.7e8 by conv3), so each conv's
    y is pre-scaled by a power-of-2 (1, 1, 2^-6, 2^-14); the fp32
    elementwise stage unscales before bias/residual/relu.  Measured
    absmax/scale error vs the fp32 reference: ~1.9e-3 (gate 2e-2).
  * Per conn chunk the y-slice weights load once and 2 (or 4) matmuls
    accumulate into separate psum banks.  tile_legalize splits each fp16
    matmul into LDWEIGHTS+MATMUL; a post-schedule strip pass removes the
    redundant (same-AP, syncless) LDWEIGHTS.
  * Split-RS pipeline: convs 1-3 run the big matmul in two column-half
    passes (A = cols 0..1023, B = 1024..2047), each followed by its own
    half-size fp16 ReduceScatter.  RS_A overlaps pass B; RS_B overlaps the
    next conv's first 12 chunks (which only need y rows from the ew-A
    half).  conv0 keeps a single 4-bank pass (it is paced by the conn DMA
    stream anyway) followed by both RS.
"""

import os

import numpy as np

import concourse.bacc as bacc
import concourse.bass as bass
import concourse.mybir as mybir
import concourse.tile as tile
from concourse.bass_utils import run_bass_kernel_spmd

A, D, F, NCORES = 2048, 64, 12, 8
NS = A // NCORES          # neighbors per core = 256
HB = NS // 2              # half-block = 128 columns owned per RS half
KL = NS * F               # local contraction size = 3072
NCH = KL // 128           # k-chunks of 128 = 24
ABLK = 512                # output free-dim block (psum bank)
NAB = A // ABLK           # 4
FO = F * D                # 768 = y columns per layer

FP = mybir.dt.float32
F16 = mybir.dt.float16

# per-conv y scales (power of 2): keep fp16-cast activations in range
SCALES = [1.0, 1.0, 2.0 ** -6, 2.0 ** -14]

STRIP = os.environ.get("CHEM_STRIP", "1") == "1"
COLL = os.environ.get("CHEM_COLL", "a2a")   # "a2a" | "rs"

_CACHE = {}

_PE = mybir.EngineType.PE
ACT_COPY = mybir.ActivationFunctionType.Copy


def _strip_redundant_ldweights(nc):
    """Remove LDWEIGHTS that reload the already-loaded stationary AP.

    tile_legalize splits every non-f32 InstMatmult into InstLdweights +
    non-self-loading InstMatmult.  Consecutive matmuls that share weights
    get one redundant load per matmul; those extra loads carry no sync
    info and can be dropped before nc.compile() (whose
    move_matmul_waits_to_ldweights pass then attaches matmul waits to the
    surviving loads)."""
    removed = 0
    for f in nc.m.functions:
        for blk in f.blocks:
            cur_ap = None
            kept = []
            for inst in blk.instructions:
                tn = type(inst).__name__
                if getattr(inst, "engine", None) == _PE:
                    if tn == "InstLdweights":
                        ap = str(inst.ins[0])
                        si = inst.sync_info
                        clean = si is None or (not si.on_wait and not si.on_update)
                        if clean and ap == cur_ap:
                            removed += 1
                            continue
                        cur_ap = ap
                    elif tn == "InstMatmult":
                        if inst.ldweights is not False:
                            cur_ap = None  # self-loading matmul clobbers PE
                    elif tn in ("InstEventSemaphore", "InstDrain", "InstISA",
                                "InstTensorLoad", "InstTensorSave"):
                        pass
                    else:
                        cur_ap = None
                kept.append(inst)
            if removed:
                blk.instructions = kept
    return removed


def _build():
    nc = bacc.Bacc("TRN2", target_bir_lowering=False, debug=False, num_devices=NCORES)

    conn_t_d = nc.dram_tensor("conn_t", [KL, A], F16, kind="ExternalInput").ap()
    xoT_d = nc.dram_tensor("xoT_sh", [D, NS], FP, kind="ExternalInput").ap()
    xoT16_d = nc.dram_tensor("xoT16_sh", [D, NS], F16, kind="ExternalInput").ap()
    fw_d = nc.dram_tensor("fw16", [D, 2 * FO], F16, kind="ExternalInput").ap()
    fb_d = nc.dram_tensor("fb", [2 * F, 2 * D], FP, kind="ExternalInput").ap()
    bondT_d = nc.dram_tensor("bondT_sh", [2 * F, NS], FP, kind="ExternalInput").ap()
    out_d = nc.dram_tensor("out_sh", [D, NS], FP, kind="ExternalOutput").ap()

    with tile.TileContext(nc) as tc:
        with (
            tc.tile_pool(name="res", bufs=1) as res_pool,
            tc.tile_pool(name="sb", bufs=1) as sb,
            tc.tile_pool(name="ypool", bufs=2) as ypool,
            tc.tile_pool(name="ztpool", bufs=4) as ztpool,
            tc.tile_pool(name="work", bufs=2) as work,
            tc.tile_pool(name="psy", bufs=2, space="PSUM") as psy,
            tc.tile_pool(name="psz", bufs=1, space="PSUM") as psz,
            tc.tile_pool(name="dram", bufs=1, space="DRAM") as dram,
        ):
            # ---- setup: small tensors first (ACT HWDGE ring), then conn
            # chunks split across the SP + POOL rings so conv0 can start
            # while conn streams in ----
            xoT16_sb = sb.tile([D, NS], F16, name="xoT16_sb", tag="xoT16_sb")
            nc.scalar.dma_start(xoT16_sb[:], xoT16_d)
            fw_sb = sb.tile([D, 2 * FO], F16, name="fw_sb", tag="fw_sb")
            nc.scalar.dma_start(fw_sb[:], fw_d)
            xoT_sb = sb.tile([D, NS], FP, name="xoT_sb", tag="xoT_sb")
            nc.scalar.dma_start(xoT_sb[:], xoT_d)
            fb_sb = sb.tile([2 * F, 2 * D], FP, name="fb_sb", tag="fb_sb")
            nc.scalar.dma_start(fb_sb[:], fb_d)
            bondT_sb = sb.tile([2 * F, NS], FP, name="bondT_sb", tag="bondT_sb")
            nc.scalar.dma_start(bondT_sb[:], bondT_d)

            conn_res = []
            for r in range(NCH):
                t = res_pool.tile([128, A], F16, name=f"connsb{r}", tag=f"connsb{r}")
                eng = nc.gpsimd if r % 2 == 1 else nc.sync
                eng.dma_start(t[:], conn_t_d[r * 128:(r + 1) * 128, :])
                conn_res.append(t)

            # per-layer bias shard: bias[l][o, a_local] (fp32, true scale)
            bias_sb = sb.tile([D, 2, NS], FP, name="bias_sb", tag="bias_sb")
            for layer in range(2):
                pb = psy.tile([D, NS], FP, name="pb", tag="py")
                nc.tensor.matmul(
                    pb[:], fb_sb[:, layer * D:(layer + 1) * D], bondT_sb[:],
                    start=True, stop=True,
                )
                nc.vector.tensor_copy(bias_sb[:, layer, :], pb[:])

            cc_in = {}
            cc_out = {}
            for i in range(4):
                for hf in "AB":
                    cc_in[i, hf] = dram.tile(
                        [NCORES, D, HB], F16, name=f"cc_in{hf}{i}", tag=f"cc_in{hf}{i}")
                    out_shape = [NCORES, D, HB] if COLL == "a2a" else [D, HB]
                    cc_out[i, hf] = dram.tile(
                        out_shape, F16, name=f"cc_out{hf}{i}", tag=f"cc_out{hf}{i}")

            scope = nc.named_scope

            def y_mm(y_sb, cur16, layer, ns_):
                """y rows for n-block ns_ (both fo-halves)."""
                for h in range(2):
                    py = psy.tile([128, FO // 2], FP, name="py", tag="py")
                    nc.tensor.matmul(
                        py[:],
                        cur16[:, ns_ * 128:(ns_ + 1) * 128],
                        fw_sb[:, layer * FO + h * (FO // 2):
                              layer * FO + (h + 1) * (FO // 2)],
                        start=True, stop=True,
                    )
                    nc.vector.tensor_copy(
                        y_sb[:, ns_, h * (FO // 2):(h + 1) * (FO // 2)], py[:]
                    )

            def drain(conv, hf, pz0, pz1):
                """Cast two psum banks (one column half) and DMA to cc_in."""
                zts = []
                for bi, pzb in enumerate((pz0, pz1)):
                    zt = ztpool.tile([D, ABLK], F16, name="zt", tag="zt")
                    if bi == 0:
                        nc.vector.tensor_copy(zt[:], pzb[:])
                    else:
                        nc.scalar.activation(zt[:], pzb[:], ACT_COPY)
                    zts.append(zt)
                for bi, zt in enumerate(zts):
                    for j in range(4):
                        eng = nc.scalar if j % 2 == 0 else nc.sync
                        eng.dma_start(
                            cc_in[conv, hf][4 * bi + j, :, :],
                            zt[:, j * HB:(j + 1) * HB],
                        )

            def rs(conv, hf):
                scc = scope(f"cc{conv}{hf}"); scc.__enter__()
                if COLL == "a2a":
                    # direct mesh exchange; block j received = core j's
                    # partial z for OUR columns.  Summed locally afterwards.
                    nc.gpsimd.collective_compute(
                        "AllToAll",
                        mybir.AluOpType.bypass,
                        replica_groups=[list(range(NCORES))],
                        ins=[cc_in[conv, hf].opt()],
                        outs=[cc_out[conv, hf].opt()],
                    )
                else:
                    nc.gpsimd.collective_compute(
                        "ReduceScatter",
                        mybir.AluOpType.add,
                        replica_groups=[list(range(NCORES))],
                        ins=[cc_in[conv, hf].opt()],
                        outs=[cc_out[conv, hf].opt()],
                    )
                scc.__exit__(None, None, None)

            def elementwise(conv, hf, layer, nxt, c16):
                """(a2a: local 8-block sum) + unscale + bias + residual +
                relu for one 128-col half."""
                hh = 0 if hf == "A" else 1
                hs = slice(hh * HB, (hh + 1) * HB)
                inv_s = 1.0 / SCALES[conv]
                if COLL == "a2a":
                    sl = work.tile([D, NCORES, HB], F16,
                                   name=f"sl{hf}", tag=f"sl{hf}")
                    for j in range(NCORES):
                        eng = nc.scalar if j % 2 == 0 else nc.sync
                        eng.dma_start(sl[:, j, :], cc_out[conv, hf][j, :, :])
                    acc = work.tile([D, HB], F16, name=f"acc{hf}", tag=f"acc{hf}")
                    nc.vector.tensor_add(acc[:], sl[:, 0, :], sl[:, 1, :])
                    for j in range(2, NCORES):
                        nc.vector.tensor_add(acc[:], acc[:], sl[:, j, :])
                    sl = acc
                else:
                    sl = work.tile([D, HB], F16, name=f"sl{hf}", tag=f"sl{hf}")
                    nc.scalar.dma_start(sl[:], cc_out[conv, hf])
                t1 = work.tile([D, HB], FP, name=f"t1{hf}", tag=f"t1{hf}")
                nc.vector.tensor_scalar_mul(t1[:], sl[:], inv_s)
                t2 = work.tile([D, HB], FP, name=f"t2{hf}", tag=f"t2{hf}")
                nc.vector.tensor_add(t2[:], t1[:], bias_sb[:, layer, hs])
                if conv % 2 == 1:
                    t3 = work.tile([D, HB], FP, name=f"t3{hf}", tag=f"t3{hf}")
                    nc.vector.tensor_add(t3[:], t2[:], xoT_sb[:, hs])
                    t2 = t3
                nc.vector.tensor_scalar_max(nxt[:, hs], t2[:], 0.0)
                if conv < 3:
                    nc.vector.tensor_scalar_mul(
                        c16[:, hs], nxt[:, hs], SCALES[conv + 1])
                else:
                    nc.scalar.dma_start(out_d[:, hs], nxt[:, hs])

            # ---------------- conv 0: single pass, paced by conn DMA -------
            sc = scope("conv0"); sc.__enter__()
            y_sb = ypool.tile([128, 2, FO], F16, name="y_sb", tag="y_sb")
            y_mm(y_sb, xoT16_sb, 0, 0)
            y_mm(y_sb, xoT16_sb, 0, 1)
            pz = [psz.tile([D, ABLK], FP, name="pz", tag=f"pz{t_}")
                  for t_ in ("A0", "A1", "B0", "B1")]
            for r in range(NCH):
                f_, ns_ = r // 2, r % 2
                lhsT = y_sb[:, ns_, f_ * D:(f_ + 1) * D]
                for ab in range(NAB):
                    nc.tensor.matmul(
                        pz[ab][:], lhsT,
                        conn_res[r][:, ab * ABLK:(ab + 1) * ABLK],
                        start=(r == 0), stop=(r == NCH - 1),
                    )
            drain(0, "A", pz[0], pz[1])
            sc.__exit__(None, None, None)
            rs(0, "A")
            sc = scope("conv0d"); sc.__enter__()
            drain(0, "B", pz[2], pz[3])
            sc.__exit__(None, None, None)
            rs(0, "B")

            # ---------------- convs 1-3: two passes + split RS -------------
            prev_nxt = work.tile([D, NS], FP, name="nxt", tag="nxt")
            prev_c16 = work.tile([D, NS], F16, name="c16", tag="c16")
            elementwise(0, "A", 0, prev_nxt, prev_c16)

            for conv in range(1, 4):
                layer = conv // 2
                sc = scope(f"conv{conv}"); sc.__enter__()
                y_sb = ypool.tile([128, 2, FO], F16, name="y_sb", tag="y_sb")
                y_mm(y_sb, prev_c16, layer, 0)

                pzA = [psz.tile([D, ABLK], FP, name="pz", tag=f"pzA{b_}")
                       for b_ in range(2)]
                pzB = [psz.tile([D, ABLK], FP, name="pz", tag=f"pzB{b_}")
                       for b_ in range(2)]

                # pass A, chunks with ns=0 first (only need y half 0)
                for ci, (ns_, f_) in enumerate(
                        [(n, f) for n in range(2) for f in range(F)]):
                    if ci == F:
                        # before the ns=1 chunks: finish the previous conv's
                        # second half + this conv's y rows for n-block 1
                        elementwise(conv - 1, "B", (conv - 1) // 2,
                                    prev_nxt, prev_c16)
                        y_mm(y_sb, prev_c16, layer, 1)
                    r = 2 * f_ + ns_
                    lhsT = y_sb[:, ns_, f_ * D:(f_ + 1) * D]
                    for ab in range(2):
                        nc.tensor.matmul(
                            pzA[ab][:], lhsT,
                            conn_res[r][:, ab * ABLK:(ab + 1) * ABLK],
                            start=(ci == 0), stop=(ci == NCH - 1),
                        )
                drain(conv, "A", pzA[0], pzA[1])
                sc.__exit__(None, None, None)
                rs(conv, "A")

                sc = scope(f"conv{conv}b"); sc.__enter__()
                # pass B
                for ci, (ns_, f_) in enumerate(
                        [(n, f) for n in range(2) for f in range(F)]):
                    r = 2 * f_ + ns_
                    lhsT = y_sb[:, ns_, f_ * D:(f_ + 1) * D]
                    for ab in range(2):
                        nc.tensor.matmul(
                            pzB[ab][:], lhsT,
                            conn_res[r][:, (2 + ab) * ABLK:(3 + ab) * ABLK],
                            start=(ci == 0), stop=(ci == NCH - 1),
                        )
                drain(conv, "B", pzB[0], pzB[1])
                sc.__exit__(None, None, None)
                rs(conv, "B")

                nxt = work.tile([D, NS], FP, name="nxt", tag="nxt")
                c16 = (work.tile([D, NS], F16, name="c16", tag="c16")
                       if conv < 3 else None)
                elementwise(conv, "A", layer, nxt, c16)
                prev_nxt, prev_c16 = nxt, c16

            # tail: second half of conv3
            elementwise(3, "B", 1, prev_nxt, None)

    if STRIP:
        n = _strip_redundant_ldweights(nc)
        # conv0: 24 chunks x 3; convs 1-3: 48 chunks x 1 each = 216 total,
        # minus the few that carry sync waits and must stay
        # (+8: the two fo-halves of each y_mm call share lhsT weights)
        expect = NCH * 3 + 3 * 2 * NCH + 8
        assert expect - 24 <= n <= expect, f"stripped {n} ldweights"
    nc.compile()
    return nc


def _get_nc():
    if "nc" not in _CACHE:
        _CACHE["nc"] = _build()
    return _CACHE["nc"]


def _own_idx(c):
    return np.r_[c * HB:(c + 1) * HB, A // 2 + c * HB:A // 2 + (c + 1) * HB]


def _prep_in_maps(node_property_tensor, connectivity_tensor, bond_property_tensor,
                  filters0, filters1):
    x = np.ascontiguousarray(node_property_tensor, dtype=np.float32)
    conn = np.ascontiguousarray(connectivity_tensor, dtype=np.float32)
    bond = np.ascontiguousarray(bond_property_tensor, dtype=np.float32)
    f0 = np.ascontiguousarray(filters0, dtype=np.float32)
    f1 = np.ascontiguousarray(filters1, dtype=np.float32)

    # host-side layout transforms (pure transpose/reshape/slice/cast)
    xT = np.ascontiguousarray(x.T)                                   # [D, A]
    xT16 = (xT * SCALES[0]).astype(np.float16)
    fw = np.concatenate(
        [f[:, :, :D].transpose(2, 1, 0).reshape(D, FO) for f in (f0, f1)], axis=1
    ).astype(np.float16)                                             # [D, 2*FO]
    fw = np.ascontiguousarray(fw)
    fb = np.concatenate(
        [f[:, :, D:].reshape(D, 2 * F).T for f in (f0, f1)], axis=1
    )                                                                # [2F, 2D]
    fb = np.ascontiguousarray(fb)
    bondT = np.ascontiguousarray(bond.transpose(1, 2, 0).reshape(2 * F, A))
    conn16 = conn.astype(np.float16)

    in_maps = []
    for c in range(NCORES):
        idx = _own_idx(c)
        conn_t = np.ascontiguousarray(
            conn16[:, idx, :].transpose(2, 1, 0).reshape(KL, A)
        )
        in_maps.append({
            "conn_t": conn_t,
            "xoT_sh": np.ascontiguousarray(xT[:, idx]),
            "xoT16_sh": np.ascontiguousarray(xT16[:, idx]),
            "fw16": fw,
            "fb": fb,
            "bondT_sh": np.ascontiguousarray(bondT[:, idx]),
        })
    return in_maps


def kernel(node_property_tensor, connectivity_tensor, bond_property_tensor,
           filters0, filters1):
    in_maps = _prep_in_maps(node_property_tensor, connectivity_tensor,
                            bond_property_tensor, filters0, filters1)
    nc = _get_nc()
    res = run_bass_kernel_spmd(nc, in_maps, core_ids=list(range(NCORES)))
    outT = np.empty((D, A), dtype=np.float32)
    for c in range(NCORES):
        outT[:, _own_idx(c)] = res.results[c]["out_sh"]
    return np.ascontiguousarray(outT.T)


def run_traced(in_maps, stitch=False):
    """For test.py: run with NTFF tracing, return BassKernelResults."""
    kw = {}
    if stitch:
        kw = dict(trace_cores=list(range(NCORES)), stitch_traces=True)
    return run_bass_kernel_spmd(
        _get_nc(), in_maps, core_ids=list(range(NCORES)), trace=True, **kw
    )


def make_in_maps(**inputs):
    """Expose the host-side prep for test.py tracing path."""
    return _prep_in_maps(
        inputs["node_property_tensor"], inputs["connectivity_tensor"],
        inputs["bond_property_tensor"], inputs["filters0"], inputs["filters1"])


# revision 12
# speedup vs baseline: 1.0733x; 1.0533x over previous
"""Trainium2 Bass kernel for nn_ChemResBlock (gnn_message_passing).

Reference computation (A=2048 atoms, D=64 depth, F=12 filter slots):
    chemconv(x)[a,o] = sum_{n,f,d} conn[a,n,f] * x[n,d] * filters[o,f,d]
                       + sum_{f,c} bond[a,f,c] * filters[o,f,D+c]
    for filt in (f0, f1):
        out = relu(chemconv(out)); out = chemconv(out); out = relu(out + x)

Kernel strategy (8 NeuronCores):
  * Contract-reorder: out[a,o] = sum_{n,f} conn[a,n,f] * y[n,f,o] with
    y[n,f,o] = sum_d x[n,d]*filters[o,f,d]  (tiny per-shard precompute), so
    the big conn tensor is consumed by plain [128,64]x[128,512] matmuls.
  * Shard the contraction (neighbor) dim n across 8 cores.  Core c owns the
    non-contiguous atom set {c*128..(c+1)*128} u {1024+c*128..1024+(c+1)*128}
    so each half of the output columns can be Reduce-Scattered separately.
  * fp16 everywhere on the big path: conn is cast to fp16 on the host
    (12 MiB/core, ALL 24 k-chunks SBUF-resident, read from HBM once, split
    across two DMA rings), y is fp16, the ReduceScatter payload is fp16.
    Activations grow ~200x per conv (absmax 5.7e8 by conv3), so each conv's
    y is pre-scaled by a power-of-2 (1, 1, 2^-6, 2^-14); the fp32
    elementwise stage unscales before bias/residual/relu.  Measured
    absmax/scale error vs the fp32 reference: ~1.9e-3 (gate 2e-2).
  * Per conn chunk the y-slice weights load once and 2 (or 4) matmuls
    accumulate into separate psum banks.  tile_legalize splits each fp16
    matmul into LDWEIGHTS+MATMUL; a post-schedule strip pass removes the
    redundant (same-AP, syncless) LDWEIGHTS.
  * Split-RS pipeline: convs 1-3 run the big matmul in two column-half
    passes (A = cols 0..1023, B = 1024..2047), each followed by its own
    half-size fp16 ReduceScatter.  RS_A overlaps pass B; RS_B overlaps the
    next conv's first 12 chunks (which only need y rows from the ew-A
    half).  conv0 keeps a single 4-bank pass (it is paced by the conn DMA
    stream anyway) followed by both RS.
"""

import os

import numpy as np

import concourse.bacc as bacc
import concourse.bass as bass
import concourse.mybir as mybir
import concourse.tile as tile
from concourse.bass_utils import run_bass_kernel_spmd

A, D, F, NCORES = 2048, 64, 12, 8
NS = A // NCORES          # neighbors per core = 256
HB = NS // 2              # half-block = 128 columns owned per RS half
KL = NS * F               # local contraction size = 3072
NCH = KL // 128           # k-chunks of 128 = 24
ABLK = 512                # output free-dim block (psum bank)
NAB = A // ABLK           # 4
FO = F * D                # 768 = y columns per layer

FP = mybir.dt.float32
F16 = mybir.dt.float16

# per-conv y scales (power of 2): keep fp16-cast activations in range
SCALES = [1.0, 1.0, 2.0 ** -6, 2.0 ** -14]

STRIP = os.environ.get("CHEM_STRIP", "1") == "1"
COLL = os.environ.get("CHEM_COLL", "a2a")   # "a2a" | "rs"

_CACHE = {}

_PE = mybir.EngineType.PE
ACT_COPY = mybir.ActivationFunctionType.Copy


def _strip_redundant_ldweights(nc):
    """Remove LDWEIGHTS that reload the already-loaded stationary AP.

    tile_legalize splits every non-f32 InstMatmult into InstLdweights +
    non-self-loading InstMatmult.  Consecutive matmuls that share weights
    get one redundant load per matmul; those extra loads carry no sync
    info and can be dropped before nc.compile() (whose
    move_matmul_waits_to_ldweights pass then attaches matmul waits to the
    surviving loads)."""
    removed = 0
    for f in nc.m.functions:
        for blk in f.blocks:
            cur_ap = None
            kept = []
            for inst in blk.instructions:
                tn = type(inst).__name__
                if getattr(inst, "engine", None) == _PE:
                    if tn == "InstLdweights":
                        ap = str(inst.ins[0])
                        si = inst.sync_info
                        clean = si is None or (not si.on_wait and not si.on_update)
                        if clean and ap == cur_ap:
                            removed += 1
                            continue
                        cur_ap = ap
                    elif tn == "InstMatmult":
                        if inst.ldweights is not False:
                            cur_ap = None  # self-loading matmul clobbers PE
                    elif tn in ("InstEventSemaphore", "InstDrain", "InstISA",
                                "InstTensorLoad", "InstTensorSave"):
                        pass
                    else:
                        cur_ap = None
                kept.append(inst)
            if removed:
                blk.instructions = kept
    return removed


def _build():
    nc = bacc.Bacc("TRN2", target_bir_lowering=False, debug=False, num_devices=NCORES)

    conn_t_d = nc.dram_tensor("conn_t", [KL, A], F16, kind="ExternalInput").ap()
    xoT_d = nc.dram_tensor("xoT_sh", [D, NS], FP, kind="ExternalInput").ap()
    xoT16_d = nc.dram_tensor("xoT16_sh", [D, NS], F16, kind="ExternalInput").ap()
    fw_d = nc.dram_tensor("fw16", [D, 2 * FO], F16, kind="ExternalInput").ap()
    fb_d = nc.dram_tensor("fb", [2 * F, 2 * D], FP, kind="ExternalInput").ap()
    bondT_d = nc.dram_tensor("bondT_sh", [2 * F, NS], FP, kind="ExternalInput").ap()
    out_d = nc.dram_tensor("out_sh", [D, NS], FP, kind="ExternalOutput").ap()

    with tile.TileContext(nc) as tc:
        with (
            tc.tile_pool(name="res", bufs=1) as res_pool,
            tc.tile_pool(name="sb", bufs=1) as sb,
            tc.tile_pool(name="ypool", bufs=2) as ypool,
            tc.tile_pool(name="ztpool", bufs=4) as ztpool,
            tc.tile_pool(name="work", bufs=2) as work,
            tc.tile_pool(name="psy", bufs=2, space="PSUM") as psy,
            tc.tile_pool(name="psz", bufs=1, space="PSUM") as psz,
            tc.tile_pool(name="dram", bufs=1, space="DRAM") as dram,
        ):
            # ---- setup: small tensors first (ACT HWDGE ring), then conn
            # chunks split across the SP + POOL rings so conv0 can start
            # while conn streams in ----
            xoT16_sb = sb.tile([D, NS], F16, name="xoT16_sb", tag="xoT16_sb")
            nc.scalar.dma_start(xoT16_sb[:], xoT16_d)
            fw_sb = sb.tile([D, 2 * FO], F16, name="fw_sb", tag="fw_sb")
            nc.scalar.dma_start(fw_sb[:], fw_d)
            xoT_sb = sb.tile([D, NS], FP, name="xoT_sb", tag="xoT_sb")
            nc.scalar.dma_start(xoT_sb[:], xoT_d)
            fb_sb = sb.tile([2 * F, 2 * D], FP, name="fb_sb", tag="fb_sb")
            nc.scalar.dma_start(fb_sb[:], fb_d)
            bondT_sb = sb.tile([2 * F, NS], FP, name="bondT_sb", tag="bondT_sb")
            nc.scalar.dma_start(bondT_sb[:], bondT_d)

            conn_res = []
            for r in range(NCH):
                t = res_pool.tile([128, A], F16, name=f"connsb{r}", tag=f"connsb{r}")
                eng = nc.gpsimd if r % 2 == 1 else nc.sync
                eng.dma_start(t[:], conn_t_d[r * 128:(r + 1) * 128, :])
                conn_res.append(t)

            # per-layer bias shard: bias[l][o, a_local] (fp32, true scale)
            bias_sb = sb.tile([D, 2, NS], FP, name="bias_sb", tag="bias_sb")
            for layer in range(2):
                pb = psy.tile([D, NS], FP, name="pb", tag="py")
                nc.tensor.matmul(
                    pb[:], fb_sb[:, layer * D:(layer + 1) * D], bondT_sb[:],
                    start=True, stop=True,
                )
                nc.vector.tensor_copy(bias_sb[:, layer, :], pb[:])

            cc_in = {}
            cc_out = {}
            for i in range(4):
                for hf in "AB":
                    cc_in[i, hf] = dram.tile(
                        [NCORES, D, HB], F16, name=f"cc_in{hf}{i}", tag=f"cc_in{hf}{i}")
                    out_shape = [NCORES, D, HB] if COLL == "a2a" else [D, HB]
                    cc_out[i, hf] = dram.tile(
                        out_shape, F16, name=f"cc_out{hf}{i}", tag=f"cc_out{hf}{i}")

            scope = nc.named_scope

            def y_mm(y_sb, cur16, layer, ns_):
                """y rows for n-block ns_ (both fo-halves)."""
                for h in range(2):
                    py = psy.tile([128, FO // 2], FP, name="py", tag="py")
                    nc.tensor.matmul(
                        py[:],
                        cur16[:, ns_ * 128:(ns_ + 1) * 128],
                        fw_sb[:, layer * FO + h * (FO // 2):
                              layer * FO + (h + 1) * (FO // 2)],
                        start=True, stop=True,
                    )
                    nc.vector.tensor_copy(
                        y_sb[:, ns_, h * (FO // 2):(h + 1) * (FO // 2)], py[:]
                    )

            def drain(conv, hf, pz0, pz1):
                """Cast two psum banks (one column half) and DMA to cc_in."""
                zts = []
                for bi, pzb in enumerate((pz0, pz1)):
                    zt = ztpool.tile([D, ABLK], F16, name="zt", tag="zt")
                    if bi == 0:
                        nc.vector.tensor_copy(zt[:], pzb[:])
                    else:
                        nc.scalar.activation(zt[:], pzb[:], ACT_COPY)
                    zts.append(zt)
                for bi, zt in enumerate(zts):
                    for j in range(4):
                        eng = nc.scalar if j % 2 == 0 else nc.sync
                        eng.dma_start(
                            cc_in[conv, hf][4 * bi + j, :, :],
                            zt[:, j * HB:(j + 1) * HB],
                        )

            def rs(conv, hf):
                scc = scope(f"cc{conv}{hf}"); scc.__enter__()
                if COLL == "a2a":
                    # direct mesh exchange; block j received = core j's
                    # partial z for OUR columns.  Summed locally afterwards.
                    nc.gpsimd.collective_compute(
                        "AllToAll",
                        mybir.AluOpType.bypass,
                        replica_groups=[list(range(NCORES))],
                        ins=[cc_in[conv, hf].opt()],
                        outs=[cc_out[conv, hf].opt()],
                    )
                else:
                    nc.gpsimd.collective_compute(
                        "ReduceScatter",
                        mybir.AluOpType.add,
                        replica_groups=[list(range(NCORES))],
                        ins=[cc_in[conv, hf].opt()],
                        outs=[cc_out[conv, hf].opt()],
                    )
                scc.__exit__(None, None, None)

            def elementwise(conv, hf, layer, nxt, c16):
                """(a2a: local 8-block sum) + unscale + bias + residual +
                relu for one 128-col half."""
                hh = 0 if hf == "A" else 1
                hs = slice(hh * HB, (hh + 1) * HB)
                inv_s = 1.0 / SCALES[conv]
                if COLL == "a2a":
                    sl = work.tile([D, NCORES, HB], F16,
                                   name=f"sl{hf}", tag=f"sl{hf}")
                    for j in range(NCORES):
                        eng = nc.scalar if j % 2 == 0 else nc.sync
                        eng.dma_start(sl[:, j, :], cc_out[conv, hf][j, :, :])
                    # fp32 tree reduction: one fp16 rounding per partial only
                    p = []
                    for j in range(NCORES // 2):
                        t = work.tile([D, HB], FP, name=f"p{hf}{j}",
                                      tag=f"p{hf}{j}")
                        nc.vector.tensor_add(
                            t[:], sl[:, 2 * j, :], sl[:, 2 * j + 1, :])
                        p.append(t)
                    q = []
                    for j in range(2):
                        t = work.tile([D, HB], FP, name=f"q{hf}{j}",
                                      tag=f"q{hf}{j}")
                        nc.vector.tensor_add(t[:], p[2 * j][:], p[2 * j + 1][:])
                        q.append(t)
                    tot = work.tile([D, HB], FP, name=f"tot{hf}", tag=f"tot{hf}")
                    nc.vector.tensor_add(tot[:], q[0][:], q[1][:])
                else:
                    sl16 = work.tile([D, HB], F16, name=f"sl{hf}", tag=f"sl{hf}")
                    nc.scalar.dma_start(sl16[:], cc_out[conv, hf])
                    tot = sl16
                t1 = work.tile([D, HB], FP, name=f"t1{hf}", tag=f"t1{hf}")
                nc.vector.tensor_scalar_mul(t1[:], tot[:], inv_s)
                t2 = work.tile([D, HB], FP, name=f"t2{hf}", tag=f"t2{hf}")
                nc.vector.tensor_add(t2[:], t1[:], bias_sb[:, layer, hs])
                if conv % 2 == 1:
                    t3 = work.tile([D, HB], FP, name=f"t3{hf}", tag=f"t3{hf}")
                    nc.vector.tensor_add(t3[:], t2[:], xoT_sb[:, hs])
                    t2 = t3
                nc.vector.tensor_scalar_max(nxt[:, hs], t2[:], 0.0)
                if conv < 3:
                    nc.vector.tensor_scalar_mul(
                        c16[:, hs], nxt[:, hs], SCALES[conv + 1])
                else:
                    nc.scalar.dma_start(out_d[:, hs], nxt[:, hs])

            # ---------------- conv 0: single pass, paced by conn DMA -------
            sc = scope("conv0"); sc.__enter__()
            y_sb = ypool.tile([128, 2, FO], F16, name="y_sb", tag="y_sb")
            y_mm(y_sb, xoT16_sb, 0, 0)
            y_mm(y_sb, xoT16_sb, 0, 1)
            pz = [psz.tile([D, ABLK], FP, name="pz", tag=f"pz{t_}")
                  for t_ in ("A0", "A1", "B0", "B1")]
            for r in range(NCH):
                f_, ns_ = r // 2, r % 2
                lhsT = y_sb[:, ns_, f_ * D:(f_ + 1) * D]
                for ab in range(NAB):
                    nc.tensor.matmul(
                        pz[ab][:], lhsT,
                        conn_res[r][:, ab * ABLK:(ab + 1) * ABLK],
                        start=(r == 0), stop=(r == NCH - 1),
                    )
            drain(0, "A", pz[0], pz[1])
            sc.__exit__(None, None, None)
            rs(0, "A")
            sc = scope("conv0d"); sc.__enter__()
            drain(0, "B", pz[2], pz[3])
            sc.__exit__(None, None, None)
            rs(0, "B")

            # ---------------- convs 1-3: two passes + split RS -------------
            prev_nxt = work.tile([D, NS], FP, name="nxt", tag="nxt")
            prev_c16 = work.tile([D, NS], F16, name="c16", tag="c16")
            elementwise(0, "A", 0, prev_nxt, prev_c16)

            for conv in range(1, 4):
                layer = conv // 2
                sc = scope(f"conv{conv}"); sc.__enter__()
                y_sb = ypool.tile([128, 2, FO], F16, name="y_sb", tag="y_sb")
                y_mm(y_sb, prev_c16, layer, 0)

                pzA = [psz.tile([D, ABLK], FP, name="pz", tag=f"pzA{b_}")
                       for b_ in range(2)]
                pzB = [psz.tile([D, ABLK], FP, name="pz", tag=f"pzB{b_}")
                       for b_ in range(2)]

                # pass A, chunks with ns=0 first (only need y half 0)
                for ci, (ns_, f_) in enumerate(
                        [(n, f) for n in range(2) for f in range(F)]):
                    if ci == F:
                        # before the ns=1 chunks: finish the previous conv's
                        # second half + this conv's y rows for n-block 1
                        elementwise(conv - 1, "B", (conv - 1) // 2,
                                    prev_nxt, prev_c16)
                        y_mm(y_sb, prev_c16, layer, 1)
                    r = 2 * f_ + ns_
                    lhsT = y_sb[:, ns_, f_ * D:(f_ + 1) * D]
                    for ab in range(2):
                        nc.tensor.matmul(
                            pzA[ab][:], lhsT,
                            conn_res[r][:, ab * ABLK:(ab + 1) * ABLK],
                            start=(ci == 0), stop=(ci == NCH - 1),
                        )
                drain(conv, "A", pzA[0], pzA[1])
                sc.__exit__(None, None, None)
                rs(conv, "A")

                # ew-A emitted BEFORE pass B so its vector/DMA queue slots
                # precede drainB's: it executes during pass B as soon as the
                # half-A collective lands, and the next conv's y0 can start
                # immediately after pass B's last matmul.
                nxt = work.tile([D, NS], FP, name="nxt", tag="nxt")
                c16 = (work.tile([D, NS], F16, name="c16", tag="c16")
                       if conv < 3 else None)
                elementwise(conv, "A", layer, nxt, c16)

                sc = scope(f"conv{conv}b"); sc.__enter__()
                # pass B
                for ci, (ns_, f_) in enumerate(
                        [(n, f) for n in range(2) for f in range(F)]):
                    r = 2 * f_ + ns_
                    lhsT = y_sb[:, ns_, f_ * D:(f_ + 1) * D]
                    for ab in range(2):
                        nc.tensor.matmul(
                            pzB[ab][:], lhsT,
                            conn_res[r][:, (2 + ab) * ABLK:(3 + ab) * ABLK],
                            start=(ci == 0), stop=(ci == NCH - 1),
                        )
                drain(conv, "B", pzB[0], pzB[1])
                sc.__exit__(None, None, None)
                rs(conv, "B")
                prev_nxt, prev_c16 = nxt, c16

            # tail: second half of conv3
            elementwise(3, "B", 1, prev_nxt, None)

    if STRIP:
        n = _strip_redundant_ldweights(nc)
        # conv0: 24 chunks x 3; convs 1-3: 48 chunks x 1 each = 216 total,
        # minus the few that carry sync waits and must stay
        # (+8: the two fo-halves of each y_mm call share lhsT weights)
        expect = NCH * 3 + 3 * 2 * NCH + 8
        assert expect - 24 <= n <= expect, f"stripped {n} ldweights"
    nc.compile()
    return nc


def _get_nc():
    if "nc" not in _CACHE:
        _CACHE["nc"] = _build()
    return _CACHE["nc"]


def _own_idx(c):
    return np.r_[c * HB:(c + 1) * HB, A // 2 + c * HB:A // 2 + (c + 1) * HB]


def _prep_in_maps(node_property_tensor, connectivity_tensor, bond_property_tensor,
                  filters0, filters1):
    x = np.ascontiguousarray(node_property_tensor, dtype=np.float32)
    conn = np.ascontiguousarray(connectivity_tensor, dtype=np.float32)
    bond = np.ascontiguousarray(bond_property_tensor, dtype=np.float32)
    f0 = np.ascontiguousarray(filters0, dtype=np.float32)
    f1 = np.ascontiguousarray(filters1, dtype=np.float32)

    # host-side layout transforms (pure transpose/reshape/slice/cast)
    xT = np.ascontiguousarray(x.T)                                   # [D, A]
    xT16 = (xT * SCALES[0]).astype(np.float16)
    fw = np.concatenate(
        [f[:, :, :D].transpose(2, 1, 0).reshape(D, FO) for f in (f0, f1)], axis=1
    ).astype(np.float16)                                             # [D, 2*FO]
    fw = np.ascontiguousarray(fw)
    fb = np.concatenate(
        [f[:, :, D:].reshape(D, 2 * F).T for f in (f0, f1)], axis=1
    )                                                                # [2F, 2D]
    fb = np.ascontiguousarray(fb)
    bondT = np.ascontiguousarray(bond.transpose(1, 2, 0).reshape(2 * F, A))
    conn16 = conn.astype(np.float16)

    in_maps = []
    for c in range(NCORES):
        idx = _own_idx(c)
        conn_t = np.ascontiguousarray(
            conn16[:, idx, :].transpose(2, 1, 0).reshape(KL, A)
        )
        in_maps.append({
            "conn_t": conn_t,
            "xoT_sh": np.ascontiguousarray(xT[:, idx]),
            "xoT16_sh": np.ascontiguousarray(xT16[:, idx]),
            "fw16": fw,
            "fb": fb,
            "bondT_sh": np.ascontiguousarray(bondT[:, idx]),
        })
    return in_maps


def kernel(node_property_tensor, connectivity_tensor, bond_property_tensor,
           filters0, filters1):
    in_maps = _prep_in_maps(node_property_tensor, connectivity_tensor,
                            bond_property_tensor, filters0, filters1)
    nc = _get_nc()
    res = run_bass_kernel_spmd(nc, in_maps, core_ids=list(range(NCORES)))
    outT = np.empty((D, A), dtype=np.float32)
    for c in range(NCORES):
        outT[:, _own_idx(c)] = res.results[c]["out_sh"]
    return np.ascontiguousarray(outT.T)


def run_traced(in_maps, stitch=False):
    """For test.py: run with NTFF tracing, return BassKernelResults."""
    kw = {}
    if stitch:
        kw = dict(trace_cores=list(range(NCORES)), stitch_traces=True)
    return run_bass_kernel_spmd(
        _get_nc(), in_maps, core_ids=list(range(NCORES)), trace=True, **kw
    )


def make_in_maps(**inputs):
    """Expose the host-side prep for test.py tracing path."""
    return _prep_in_maps(
        inputs["node_property_tensor"], inputs["connectivity_tensor"],
        inputs["bond_property_tensor"], inputs["filters0"], inputs["filters1"])


# revision 16
# speedup vs baseline: 1.0861x; 1.0119x over previous
"""Trainium2 Bass kernel for nn_ChemResBlock (gnn_message_passing).

Reference computation (A=2048 atoms, D=64 depth, F=12 filter slots):
    chemconv(x)[a,o] = sum_{n,f,d} conn[a,n,f] * x[n,d] * filters[o,f,d]
                       + sum_{f,c} bond[a,f,c] * filters[o,f,D+c]
    for filt in (f0, f1):
        out = relu(chemconv(out)); out = chemconv(out); out = relu(out + x)

Kernel strategy (8 NeuronCores):
  * Contract-reorder: out[a,o] = sum_{n,f} conn[a,n,f] * y[n,f,o] with
    y[n,f,o] = sum_d x[n,d]*filters[o,f,d]  (tiny per-shard precompute), so
    the big conn tensor is consumed by plain [128,64]x[128,512] matmuls.
  * Shard the contraction (neighbor) dim n across 8 cores.  Core c owns the
    non-contiguous atom set {c*128..(c+1)*128} u {1024+c*128..1024+(c+1)*128}
    so each half of the output columns can be Reduce-Scattered separately.
  * fp16 everywhere on the big path: conn is cast to fp16 on the host
    (12 MiB/core, ALL 24 k-chunks SBUF-resident, read from HBM once, split
    across two DMA rings), y is fp16, the ReduceScatter payload is fp16.
    Activations grow ~200x per conv (absmax 5.7e8 by conv3), so each conv's
    y is pre-scaled by a power-of-2 (1, 1, 2^-6, 2^-14); the fp32
    elementwise stage unscales before bias/residual/relu.  Measured
    absmax/scale error vs the fp32 reference: ~1.9e-3 (gate 2e-2).
  * Per conn chunk the y-slice weights load once and 2 (or 4) matmuls
    accumulate into separate psum banks.  tile_legalize splits each fp16
    matmul into LDWEIGHTS+MATMUL; a post-schedule strip pass removes the
    redundant (same-AP, syncless) LDWEIGHTS.
  * Split-RS pipeline: convs 1-3 run the big matmul in two column-half
    passes (A = cols 0..1023, B = 1024..2047), each followed by its own
    half-size fp16 ReduceScatter.  RS_A overlaps pass B; RS_B overlaps the
    next conv's first 12 chunks (which only need y rows from the ew-A
    half).  conv0 keeps a single 4-bank pass (it is paced by the conn DMA
    stream anyway) followed by both RS.
"""

import os

import numpy as np

import concourse.bacc as bacc
import concourse.bass as bass
import concourse.mybir as mybir
import concourse.tile as tile
from concourse.bass_utils import run_bass_kernel_spmd

A, D, F, NCORES = 2048, 64, 12, 8
NS = A // NCORES          # neighbors per core = 256
HB = NS // 2              # half-block = 128 columns owned per RS half
KL = NS * F               # local contraction size = 3072
NCH = KL // 128           # k-chunks of 128 = 24
ABLK = 512                # output free-dim block (psum bank)
NAB = A // ABLK           # 4
FO = F * D                # 768 = y columns per layer

FP = mybir.dt.float32
F16 = mybir.dt.float16

# per-conv y scales (power of 2): keep fp16-cast activations in range
SCALES = [1.0, 1.0, 2.0 ** -6, 2.0 ** -14]

STRIP = os.environ.get("CHEM_STRIP", "1") == "1"
COLL = os.environ.get("CHEM_COLL", "a2a")   # "a2a" | "rs"

_CACHE = {}

_PE = mybir.EngineType.PE
ACT_COPY = mybir.ActivationFunctionType.Copy


def _strip_redundant_ldweights(nc):
    """Remove LDWEIGHTS that reload the already-loaded stationary AP.

    tile_legalize splits every non-f32 InstMatmult into InstLdweights +
    non-self-loading InstMatmult.  Consecutive matmuls that share weights
    get one redundant load per matmul; those extra loads carry no sync
    info and can be dropped before nc.compile() (whose
    move_matmul_waits_to_ldweights pass then attaches matmul waits to the
    surviving loads)."""
    removed = 0
    for f in nc.m.functions:
        for blk in f.blocks:
            cur_ap = None
            kept = []
            for inst in blk.instructions:
                tn = type(inst).__name__
                if getattr(inst, "engine", None) == _PE:
                    if tn == "InstLdweights":
                        ap = str(inst.ins[0])
                        si = inst.sync_info
                        clean = si is None or (not si.on_wait and not si.on_update)
                        if clean and ap == cur_ap:
                            removed += 1
                            continue
                        cur_ap = ap
                    elif tn == "InstMatmult":
                        if inst.ldweights is not False:
                            cur_ap = None  # self-loading matmul clobbers PE
                    elif tn in ("InstEventSemaphore", "InstDrain", "InstISA",
                                "InstTensorLoad", "InstTensorSave"):
                        pass
                    else:
                        cur_ap = None
                kept.append(inst)
            if removed:
                blk.instructions = kept
    return removed


def _build():
    nc = bacc.Bacc("TRN2", target_bir_lowering=False, debug=False, num_devices=NCORES)

    conn_t_d = nc.dram_tensor("conn_t", [KL, A], F16, kind="ExternalInput").ap()
    xoT_d = nc.dram_tensor("xoT_sh", [D, NS], FP, kind="ExternalInput").ap()
    xoT16_d = nc.dram_tensor("xoT16_sh", [D, NS], F16, kind="ExternalInput").ap()
    fw_d = nc.dram_tensor("fw16", [D, 2 * FO], F16, kind="ExternalInput").ap()
    fb_d = nc.dram_tensor("fb", [2 * F, 2 * D], FP, kind="ExternalInput").ap()
    bondT_d = nc.dram_tensor("bondT_sh", [2 * F, NS], FP, kind="ExternalInput").ap()
    out_d = nc.dram_tensor("out_sh", [D, NS], FP, kind="ExternalOutput").ap()

    with tile.TileContext(nc) as tc:
        with (
            tc.tile_pool(name="res", bufs=1) as res_pool,
            tc.tile_pool(name="sb", bufs=1) as sb,
            tc.tile_pool(name="ypool", bufs=2) as ypool,
            tc.tile_pool(name="ztpool", bufs=4) as ztpool,
            tc.tile_pool(name="work", bufs=2) as work,
            tc.tile_pool(name="psy", bufs=2, space="PSUM") as psy,
            tc.tile_pool(name="psz", bufs=1, space="PSUM") as psz,
            tc.tile_pool(name="dram", bufs=1, space="DRAM") as dram,
        ):
            # ---- setup: small tensors first (ACT HWDGE ring), then conn
            # chunks split across the SP + POOL rings so conv0 can start
            # while conn streams in ----
            xoT16_sb = sb.tile([D, NS], F16, name="xoT16_sb", tag="xoT16_sb")
            nc.scalar.dma_start(xoT16_sb[:], xoT16_d)
            fw_sb = sb.tile([D, 2 * FO], F16, name="fw_sb", tag="fw_sb")
            nc.scalar.dma_start(fw_sb[:], fw_d)
            xoT_sb = sb.tile([D, NS], FP, name="xoT_sb", tag="xoT_sb")
            nc.scalar.dma_start(xoT_sb[:], xoT_d)
            fb_sb = sb.tile([2 * F, 2 * D], FP, name="fb_sb", tag="fb_sb")
            nc.scalar.dma_start(fb_sb[:], fb_d)
            bondT_sb = sb.tile([2 * F, NS], FP, name="bondT_sb", tag="bondT_sb")
            nc.scalar.dma_start(bondT_sb[:], bondT_d)

            conn_res = []
            for r in range(NCH):
                t = res_pool.tile([128, A], F16, name=f"connsb{r}", tag=f"connsb{r}")
                eng = nc.gpsimd if r % 2 == 1 else nc.sync
                eng.dma_start(t[:], conn_t_d[r * 128:(r + 1) * 128, :])
                conn_res.append(t)

            # per-layer bias shard: bias[l][o, a_local] (fp32, true scale)
            bias_sb = sb.tile([D, 2, NS], FP, name="bias_sb", tag="bias_sb")
            for layer in range(2):
                pb = psy.tile([D, NS], FP, name="pb", tag="py")
                nc.tensor.matmul(
                    pb[:], fb_sb[:, layer * D:(layer + 1) * D], bondT_sb[:],
                    start=True, stop=True,
                )
                nc.vector.tensor_copy(bias_sb[:, layer, :], pb[:])

            cc_in = {}
            cc_out = {}
            for i in range(4):
                for hf in "AB":
                    cc_in[i, hf] = dram.tile(
                        [NCORES, D, HB], F16, name=f"cc_in{hf}{i}", tag=f"cc_in{hf}{i}")
                    out_shape = [NCORES, D, HB] if COLL == "a2a" else [D, HB]
                    cc_out[i, hf] = dram.tile(
                        out_shape, F16, name=f"cc_out{hf}{i}", tag=f"cc_out{hf}{i}")

            scope = nc.named_scope

            def y_mm(y_sb, cur16, layer, ns_):
                """y rows for n-block ns_ (both fo-halves)."""
                for h in range(2):
                    py = psy.tile([128, FO // 2], FP, name="py", tag="py")
                    nc.tensor.matmul(
                        py[:],
                        cur16[:, ns_ * 128:(ns_ + 1) * 128],
                        fw_sb[:, layer * FO + h * (FO // 2):
                              layer * FO + (h + 1) * (FO // 2)],
                        start=True, stop=True,
                    )
                    nc.vector.tensor_copy(
                        y_sb[:, ns_, h * (FO // 2):(h + 1) * (FO // 2)], py[:]
                    )

            def drain(conv, hf, pz0, pz1):
                """Cast two psum banks (one column half) and DMA to cc_in."""
                zts = []
                for bi, pzb in enumerate((pz0, pz1)):
                    zt = ztpool.tile([D, ABLK], F16, name="zt", tag="zt")
                    if bi == 0:
                        nc.vector.tensor_copy(zt[:], pzb[:])
                    else:
                        nc.scalar.activation(zt[:], pzb[:], ACT_COPY)
                    zts.append(zt)
                for bi, zt in enumerate(zts):
                    for j in range(4):
                        eng = nc.scalar if j % 2 == 0 else nc.sync
                        eng.dma_start(
                            cc_in[conv, hf][4 * bi + j, :, :],
                            zt[:, j * HB:(j + 1) * HB],
                        )

            def rs(conv, hf):
                scc = scope(f"cc{conv}{hf}"); scc.__enter__()
                if COLL == "a2a":
                    # direct mesh exchange; block j received = core j's
                    # partial z for OUR columns.  Summed locally afterwards.
                    nc.gpsimd.collective_compute(
                        "AllToAll",
                        mybir.AluOpType.bypass,
                        replica_groups=[list(range(NCORES))],
                        ins=[cc_in[conv, hf].opt()],
                        outs=[cc_out[conv, hf].opt()],
                    )
                else:
                    nc.gpsimd.collective_compute(
                        "ReduceScatter",
                        mybir.AluOpType.add,
                        replica_groups=[list(range(NCORES))],
                        ins=[cc_in[conv, hf].opt()],
                        outs=[cc_out[conv, hf].opt()],
                    )
                scc.__exit__(None, None, None)

            def elementwise(conv, hf, layer, nxt, c16):
                """(a2a: local 8-block sum) + unscale + bias + residual +
                relu for one 128-col half."""
                hh = 0 if hf == "A" else 1
                hs = slice(hh * HB, (hh + 1) * HB)
                inv_s = 1.0 / SCALES[conv]
                if COLL == "a2a":
                    sl = work.tile([D, NCORES, HB], F16,
                                   name=f"sl{hf}", tag=f"sl{hf}")
                    for j in range(NCORES):
                        eng = nc.scalar if j % 2 == 0 else nc.sync
                        eng.dma_start(sl[:, j, :], cc_out[conv, hf][j, :, :])
                    # fp32 tree reduction: one fp16 rounding per partial only
                    p = []
                    for j in range(NCORES // 2):
                        t = work.tile([D, HB], FP, name=f"p{hf}{j}",
                                      tag=f"p{hf}{j}")
                        nc.vector.tensor_add(
                            t[:], sl[:, 2 * j, :], sl[:, 2 * j + 1, :])
                        p.append(t)
                    q = []
                    for j in range(2):
                        t = work.tile([D, HB], FP, name=f"q{hf}{j}",
                                      tag=f"q{hf}{j}")
                        nc.vector.tensor_add(t[:], p[2 * j][:], p[2 * j + 1][:])
                        q.append(t)
                    tot = work.tile([D, HB], FP, name=f"tot{hf}", tag=f"tot{hf}")
                    nc.vector.tensor_add(tot[:], q[0][:], q[1][:])
                else:
                    sl16 = work.tile([D, HB], F16, name=f"sl{hf}", tag=f"sl{hf}")
                    nc.scalar.dma_start(sl16[:], cc_out[conv, hf])
                    tot = sl16
                t1 = work.tile([D, HB], FP, name=f"t1{hf}", tag=f"t1{hf}")
                nc.vector.tensor_scalar_mul(t1[:], tot[:], inv_s)
                t2 = work.tile([D, HB], FP, name=f"t2{hf}", tag=f"t2{hf}")
                nc.vector.tensor_add(t2[:], t1[:], bias_sb[:, layer, hs])
                if conv % 2 == 1:
                    t3 = work.tile([D, HB], FP, name=f"t3{hf}", tag=f"t3{hf}")
                    nc.vector.tensor_add(t3[:], t2[:], xoT_sb[:, hs])
                    t2 = t3
                nc.vector.tensor_scalar_max(nxt[:, hs], t2[:], 0.0)
                if conv < 3:
                    nc.vector.tensor_scalar_mul(
                        c16[:, hs], nxt[:, hs], SCALES[conv + 1])
                else:
                    nc.scalar.dma_start(out_d[:, hs], nxt[:, hs])

            # ---------------- conv 0: single pass, paced by conn DMA -------
            sc = scope("conv0"); sc.__enter__()
            y_sb = ypool.tile([128, 2, FO], F16, name="y_sb", tag="y_sb")
            y_mm(y_sb, xoT16_sb, 0, 0)
            y_mm(y_sb, xoT16_sb, 0, 1)
            pz = [psz.tile([D, ABLK], FP, name="pz", tag=f"pz{t_}")
                  for t_ in ("A0", "A1", "B0", "B1")]
            for r in range(NCH):
                f_, ns_ = r // 2, r % 2
                lhsT = y_sb[:, ns_, f_ * D:(f_ + 1) * D]
                for ab in range(NAB):
                    nc.tensor.matmul(
                        pz[ab][:], lhsT,
                        conn_res[r][:, ab * ABLK:(ab + 1) * ABLK],
                        start=(r == 0), stop=(r == NCH - 1),
                    )
            drain(0, "A", pz[0], pz[1])
            sc.__exit__(None, None, None)
            rs(0, "A")
            sc = scope("conv0d"); sc.__enter__()
            drain(0, "B", pz[2], pz[3])
            sc.__exit__(None, None, None)
            rs(0, "B")

            # ---------------- convs 1-3: two passes + split RS -------------
            prev_nxt = work.tile([D, NS], FP, name="nxt", tag="nxt")
            prev_c16 = work.tile([D, NS], F16, name="c16", tag="c16")
            elementwise(0, "A", 0, prev_nxt, prev_c16)

            for conv in range(1, 4):
                layer = conv // 2
                sc = scope(f"conv{conv}"); sc.__enter__()
                y_sb = ypool.tile([128, 2, FO], F16, name="y_sb", tag="y_sb")
                y_mm(y_sb, prev_c16, layer, 0)

                pzA = [psz.tile([D, ABLK], FP, name="pz", tag=f"pzA{b_}")
                       for b_ in range(2)]
                pzB = [psz.tile([D, ABLK], FP, name="pz", tag=f"pzB{b_}")
                       for b_ in range(2)]

                # pass A, chunks with ns=0 first (only need y half 0)
                for ci, (ns_, f_) in enumerate(
                        [(n, f) for n in range(2) for f in range(F)]):
                    if ci == F:
                        # before the ns=1 chunks: finish the previous conv's
                        # second half + this conv's y rows for n-block 1
                        elementwise(conv - 1, "B", (conv - 1) // 2,
                                    prev_nxt, prev_c16)
                        y_mm(y_sb, prev_c16, layer, 1)
                    r = 2 * f_ + ns_
                    lhsT = y_sb[:, ns_, f_ * D:(f_ + 1) * D]
                    for ab in range(2):
                        nc.tensor.matmul(
                            pzA[ab][:], lhsT,
                            conn_res[r][:, ab * ABLK:(ab + 1) * ABLK],
                            start=(ci == 0), stop=(ci == NCH - 1),
                        )
                drain(conv, "A", pzA[0], pzA[1])
                sc.__exit__(None, None, None)
                rs(conv, "A")

                # ew-A emitted BEFORE pass B so its vector/DMA queue slots
                # precede drainB's: it executes during pass B as soon as the
                # half-A collective lands, and the next conv's y0 can start
                # immediately after pass B's last matmul.
                nxt = work.tile([D, NS], FP, name="nxt", tag="nxt")
                c16 = (work.tile([D, NS], F16, name="c16", tag="c16")
                       if conv < 3 else None)
                elementwise(conv, "A", layer, nxt, c16)

                sc = scope(f"conv{conv}b"); sc.__enter__()
                # pass B
                for ci, (ns_, f_) in enumerate(
                        [(n, f) for n in range(2) for f in range(F)]):
                    r = 2 * f_ + ns_
                    lhsT = y_sb[:, ns_, f_ * D:(f_ + 1) * D]
                    for ab in range(2):
                        nc.tensor.matmul(
                            pzB[ab][:], lhsT,
                            conn_res[r][:, (2 + ab) * ABLK:(3 + ab) * ABLK],
                            start=(ci == 0), stop=(ci == NCH - 1),
                        )
                drain(conv, "B", pzB[0], pzB[1])
                sc.__exit__(None, None, None)
                rs(conv, "B")
                prev_nxt, prev_c16 = nxt, c16

            # tail: second half of conv3
            elementwise(3, "B", 1, prev_nxt, None)

    if STRIP:
        n = _strip_redundant_ldweights(nc)
        # conv0: 24 chunks x 3; convs 1-3: 48 chunks x 1 each = 216 total,
        # minus the few that carry sync waits and must stay
        # (+8: the two fo-halves of each y_mm call share lhsT weights)
        expect = NCH * 3 + 3 * 2 * NCH + 8
        assert expect - 24 <= n <= expect, f"stripped {n} ldweights"
    nc.compile()
    return nc


def _get_nc():
    if "nc" not in _CACHE:
        _CACHE["nc"] = _build()
    return _CACHE["nc"]


def _own_idx(c):
    return np.r_[c * HB:(c + 1) * HB, A // 2 + c * HB:A // 2 + (c + 1) * HB]


def _prep_in_maps(node_property_tensor, connectivity_tensor, bond_property_tensor,
                  filters0, filters1):
    x = np.ascontiguousarray(node_property_tensor, dtype=np.float32)
    conn = np.ascontiguousarray(connectivity_tensor, dtype=np.float32)
    bond = np.ascontiguousarray(bond_property_tensor, dtype=np.float32)
    f0 = np.ascontiguousarray(filters0, dtype=np.float32)
    f1 = np.ascontiguousarray(filters1, dtype=np.float32)

    # host-side layout transforms (pure transpose/reshape/slice/cast)
    xT = np.ascontiguousarray(x.T)                                   # [D, A]
    xT16 = (xT * SCALES[0]).astype(np.float16)
    fw = np.concatenate(
        [f[:, :, :D].transpose(2, 1, 0).reshape(D, FO) for f in (f0, f1)], axis=1
    ).astype(np.float16)                                             # [D, 2*FO]
    fw = np.ascontiguousarray(fw)
    fb = np.concatenate(
        [f[:, :, D:].reshape(D, 2 * F).T for f in (f0, f1)], axis=1
    )                                                                # [2F, 2D]
    fb = np.ascontiguousarray(fb)
    bondT = np.ascontiguousarray(bond.transpose(1, 2, 0).reshape(2 * F, A))
    conn16 = conn.astype(np.float16)

    in_maps = []
    for c in range(NCORES):
        idx = _own_idx(c)
        conn_t = np.ascontiguousarray(
            conn16[:, idx, :].transpose(2, 1, 0).reshape(KL, A)
        )
        in_maps.append({
            "conn_t": conn_t,
            "xoT_sh": np.ascontiguousarray(xT[:, idx]),
            "xoT16_sh": np.ascontiguousarray(xT16[:, idx]),
            "fw16": fw,
            "fb": fb,
            "bondT_sh": np.ascontiguousarray(bondT[:, idx]),
        })
    return in_maps


def kernel(node_property_tensor, connectivity_tensor, bond_property_tensor,
           filters0, filters1):
    in_maps = _prep_in_maps(node_property_tensor, connectivity_tensor,
                            bond_property_tensor, filters0, filters1)
    nc = _get_nc()
    res = run_bass_kernel_spmd(nc, in_maps, core_ids=list(range(NCORES)))
    outT = np.empty((D, A), dtype=np.float32)
    for c in range(NCORES):
        outT[:, _own_idx(c)] = res.results[c]["out_sh"]
    return np.ascontiguousarray(outT.T)


def run_traced(in_maps, stitch=False):
    """For test.py: run with NTFF tracing, return BassKernelResults."""
    kw = {}
    if stitch:
        kw = dict(trace_cores=list(range(NCORES)), stitch_traces=True)
    return run_bass_kernel_spmd(
        _get_nc(), in_maps, core_ids=list(range(NCORES)), trace=True, **kw
    )


def make_in_maps(**inputs):
    """Expose the host-side prep for test.py tracing path."""
    return _prep_in_maps(
        inputs["node_property_tensor"], inputs["connectivity_tensor"],
        inputs["bond_property_tensor"], inputs["filters0"], inputs["filters1"])
